# revision 52
# baseline (speedup 1.0000x reference)
"""HDGC-style GNN message passing on 8 NeuronCores via a fused Bass/Tile kernel.

Data-parallel over N: each core gets N/8=8 batches plus the full (tiny)
adjacencies and 1x1-conv weights.

Wire format (the axon tunnel is the bottleneck at ~70MB/s each way):
 - x ships as bf16 [64, 256, 64*25]  (52.4 MB)
 - the kernel returns delta = gate * bn(conv_d(A_final @ feat)) quantized to
   biased uint8 (26.2 MB) plus one f32 scale per core; the host applies
   out = relu(delta + x) in f32.

Device kernel math (per core, n=8 batches), with host-side folding:
  W_phi' = W_phi * D^-0.5 (and bias), bn folded into W_d and a bias term,
  A_base = A_prior + beta*A_2hop passed transposed, lam clipped.
  feat = x viewed [C, T*V] per batch; per position-block of 5 t's (125 rows):
    phiT/psiT  [96, pos]  = W'^T-stationary matmuls + bias
    logitsT    [125(w), 75(h,v)] = psiT-block^T @ phiT-block  (per t, h)
    e = exp(logitsT) (no max-sub: |logits| small, f32 exp is safe)
    sums_rep   [125, 75] = blockdiag_ones^T @ e  (per-row softmax denominators,
                replicated across each 25-partition block)
    A_finT     = A_baseT_rep + lam * e * recip(sums_rep)   (bf16)
    Y_h        [125(v...), wait rows=positions, cols=o] = X-block^T @ (W_d'bn)^T
    z          [125(v), 256(o)] = sum_h A_finT_h^T @ Y_h   (PSUM accum)
    gate       = sigmoid(X-block^T @ W_g^T + b_g)
    delta      = gate * (z + bnT)        (bf16, SBUF-resident)
  then: global absmax -> inv scale, PE-transpose delta to [o, pos] and emit
  uint8 = trunc(delta*inv + 128.5)  (round-half-up, in [2,254], no wrap).
"""

import threading
import numpy as np

N, C, T, V, H, O = 64, 256, 64, 25, 3, 256
D = 32
HD = H * D  # 96
BN_EPS = 1e-5
NCORES = 8
NB = N // NCORES  # batches per core

_CACHE = {}


# ---------------------------------------------------------------------------
# Device kernel (Bass/Tile)
# ---------------------------------------------------------------------------

def build_nc(nb=NB, t_dim=T, num_devices=NCORES, stages=5, use_adapt=True):
    from contextlib import ExitStack
    import concourse.bass as bass
    import concourse.mybir as mybir
    from concourse import bacc
    from concourse.tile import TileContext
    from concourse.masks import make_identity
    from concourse import bass_isa

    f32 = mybir.dt.float32
    bf16 = mybir.dt.bfloat16
    u8 = mybir.dt.uint8
    TV = t_dim * V
    # PE matmul partition offsets must be 32-aligned, so positions are packed
    # at 32-stride (25 valid + 7 pad rows per t) in groups of 4 t's = 128 rows.
    GT = 4
    assert t_dim % GT == 0
    ngroups = t_dim // GT
    TV32 = t_dim * 32

    nc = bacc.Bacc("TRN2", target_bir_lowering=False, debug=False,
                   num_devices=num_devices)

    x_bf = nc.dram_tensor("x_bf", [nb, C, TV], bf16, kind="ExternalInput")
    wphiT = nc.dram_tensor("wphiT", [H, C, D], bf16, kind="ExternalInput")
    wpsiT = nc.dram_tensor("wpsiT", [H, C, D], bf16, kind="ExternalInput")
    wgT = nc.dram_tensor("wgT", [C, O], bf16, kind="ExternalInput")
    wdT = nc.dram_tensor("wdT", [H, C, O], bf16, kind="ExternalInput")
    bphi = nc.dram_tensor("bphi", [D, H], f32, kind="ExternalInput")
    bpsi = nc.dram_tensor("bpsi", [D, H], f32, kind="ExternalInput")
    bg = nc.dram_tensor("bg", [1, O], f32, kind="ExternalInput")
    bnT = nc.dram_tensor("bnT", [1, O], f32, kind="ExternalInput")
    abaseT = nc.dram_tensor("abaseT", [V, H * V], f32, kind="ExternalInput")
    lam = nc.dram_tensor("lam", [1, 1], f32, kind="ExternalInput")

    dq = nc.dram_tensor("dq", [nb, O, TV], u8, kind="ExternalOutput")
    dmax = nc.dram_tensor("dmax", [1, 1], f32, kind="ExternalOutput")

    def bcast_rows(src_ap, parts):
        """DRAM [1, F] -> AP replicated across `parts` partitions."""
        return bass.AP(tensor=src_ap.tensor, offset=src_ap.offset,
                       ap=[[0, parts]] + list(src_ap.ap[1:]))

    with TileContext(nc) as tc, ExitStack() as ctx:
        consts = ctx.enter_context(tc.tile_pool(name="consts", bufs=1))
        xpool = ctx.enter_context(tc.tile_pool(name="xpool", bufs=2))
        projpool = ctx.enter_context(tc.tile_pool(name="projpool", bufs=2))
        gpool = ctx.enter_context(tc.tile_pool(name="gpool", bufs=3))
        dpool = ctx.enter_context(tc.tile_pool(name="dpool", bufs=1))
        qpool = ctx.enter_context(tc.tile_pool(name="qpool", bufs=2))
        # PSUM budget: 8 banks of 2KB/partition.
        # ps_a "lg": proj [96,400]f32 & logitsT [125,75]f32 share slots (2 banks)
        # ps_b "sm": sums [125,75]f32 & transpose [128,125]bf16 share (2 banks)
        # ps_y [125,3,256]f32 (2 banks), ps_g (1), ps_z (1)  -> total 8
        ps_a = ctx.enter_context(tc.tile_pool(name="ps_a", bufs=2, space="PSUM"))
        ps_b = ctx.enter_context(tc.tile_pool(name="ps_b", bufs=2, space="PSUM"))
        ps_y = ctx.enter_context(tc.tile_pool(name="ps_y", bufs=1, space="PSUM"))
        ps_g = ctx.enter_context(tc.tile_pool(name="ps_g", bufs=1, space="PSUM"))
        ps_z = ctx.enter_context(tc.tile_pool(name="ps_z", bufs=1, space="PSUM"))

        # ---- constants in SBUF ----
        # phi/psi weights are zero-padded from D=32 to 128 output columns so
        # every matmul runs in full 128x128 PE mode: interleaving tiled modes
        # (K=32 / M=32) with full-mode matmuls races on hardware (mode
        # switches need an array drain) and produced nondeterministic output.
        wphiT_sb = consts.tile([128, 2, H, 128], bf16)
        wpsiT_sb = consts.tile([128, 2, H, 128], bf16)
        nc.vector.memset(wphiT_sb[:], 0.0)
        nc.vector.memset(wpsiT_sb[:], 0.0)
        wgT_sb = consts.tile([128, 2, O], bf16)
        wdT_sb = consts.tile([128, 2, H, O], bf16)
        for k in range(2):
            nc.sync.dma_start(out=wgT_sb[:, k, :], in_=wgT[k * 128:(k + 1) * 128, :])
            for h in range(H):
                nc.sync.dma_start(out=wphiT_sb[:, k, h, 0:D],
                                  in_=wphiT[h, k * 128:(k + 1) * 128, :])
                nc.sync.dma_start(out=wpsiT_sb[:, k, h, 0:D],
                                  in_=wpsiT[h, k * 128:(k + 1) * 128, :])
                nc.sync.dma_start(out=wdT_sb[:, k, h, :],
                                  in_=wdT[h, k * 128:(k + 1) * 128, :])
        bphi_sb = consts.tile([128, H], f32)
        bpsi_sb = consts.tile([128, H], f32)
        nc.vector.memset(bphi_sb[:], 0.0)
        nc.vector.memset(bpsi_sb[:], 0.0)
        nc.sync.dma_start(out=bphi_sb[0:D, :], in_=bphi[:])
        nc.sync.dma_start(out=bpsi_sb[0:D, :], in_=bpsi[:])
        bg_sb = consts.tile([128, O], f32)
        bnT_sb = consts.tile([128, O], f32)
        nc.sync.dma_start(out=bg_sb[:], in_=bcast_rows(bg[:], 128))
        nc.sync.dma_start(out=bnT_sb[:], in_=bcast_rows(bnT[:], 128))
        abaseT_sb = consts.tile([128, H * V], f32)
        nc.vector.memset(abaseT_sb[:], 0.0)
        for j in range(GT):
            nc.sync.dma_start(out=abaseT_sb[32 * j:32 * j + V, :], in_=abaseT[:, :])
        lam_sb = consts.tile([128, 1], f32)
        nc.sync.dma_start(out=lam_sb[:], in_=bcast_rows(lam[:], 128))
        # block-diagonal ones (4 blocks of 25x25 at 32-stride)
        bdiag_sb = consts.tile([128, 128], bf16)
        nc.vector.memset(bdiag_sb[:], 0.0)
        for j in range(GT):
            nc.vector.memset(bdiag_sb[32 * j:32 * j + V, 32 * j:32 * j + V], 1.0)
        ident_sb = consts.tile([128, 128], bf16)
        make_identity(nc, ident_sb[:])

        # delta (bf16) for the whole shard + per-partition absmax stats
        delta_sb = dpool.tile([128, nb * ngroups, O], bf16)
        stats_sb = dpool.tile([128, 1], f32)
        nc.vector.memset(stats_sb[:], 0.0)

        # persistent block-diagonal e / A_finT tiles (double-buffered by group
        # parity); off-diagonal and pad entries stay zero forever, so the z
        # matmuls can run as single full-K matmuls per head with no PE tiling.
        ebd_tiles = []
        afbd_tiles = []
        for i in range(2):
            t_e = dpool.tile([128, H, 128], bf16, tag=f"ebd{i}")
            t_a = dpool.tile([128, H, 128], bf16, tag=f"afbd{i}")
            nc.vector.memset(t_e[:], 0.0)
            nc.vector.memset(t_a[:], 0.0)
            ebd_tiles.append(t_e)
            afbd_tiles.append(t_a)
        if not use_adapt:
            # debug: constant block-diagonal A (= A_baseT), no adaptive part
            abd_const = dpool.tile([128, H, 128], bf16, tag="abdc")
            nc.vector.memset(abd_const[:], 0.0)
            for tl in range(GT):
                blk = slice(32 * tl, 32 * tl + V)
                nc.vector.tensor_copy(
                    abd_const[blk, :, blk],
                    abaseT_sb[blk, :].rearrange("w (h v) -> w h v", v=V))

        CHW = min(512, TV32)
        assert TV32 % CHW == 0
        NCHUNK = TV32 // CHW

        for n in range(nb):
            # x, column-padded to 32 per t: [128, k, t, 32] (cols 25..31 zero).
            # DMA lands contiguous; the pad-strided relayout runs on DVE
            # (a strided DMA here fans out across HW queues and raced).
            x_raw = xpool.tile([128, 2, TV], bf16, tag="xraw")
            for k in range(2):
                nc.sync.dma_start(out=x_raw[:, k, :],
                                  in_=x_bf[n, k * 128:(k + 1) * 128, :])
            x_sb = xpool.tile([128, 2, t_dim, 32], bf16, tag="x")
            for k in range(2):
                nc.vector.tensor_copy(
                    x_sb[:, k, :, 0:V],
                    x_raw[:, k, :].rearrange("c (t v) -> c t v", v=V))
                nc.vector.memset(x_sb[:, k, :, V:32], 0.0)

            # phi/psi projections per head: [128(d, zero above 32), H, TV32]
            # bf16 (+bias); d^-0.5 folded into W. Rows 32-127 are zero so the
            # logits matmuls can contract over the full K=128 (full PE mode).
            phiT_sb = projpool.tile([128, H, TV32], bf16, tag="phi")
            psiT_sb = projpool.tile([128, H, TV32], bf16, tag="psi")
            xflat = x_sb[:, :, :, :].rearrange("c k t v -> c k (t v)")
            for (wT_sb, b_sb, outT) in ((wphiT_sb, bphi_sb, phiT_sb),
                                        (wpsiT_sb, bpsi_sb, psiT_sb)):
                for h in range(H):
                    for ci in range(NCHUNK):
                        ps = ps_a.tile([128, CHW], mybir.dt.float32, tag="lg")
                        for k in range(2):
                            nc.tensor.matmul(ps[:], wT_sb[:, k, h, :],
                                             xflat[:, k, ci * CHW:(ci + 1) * CHW],
                                             start=(k == 0), stop=(k == 1))
                        # rows D..127 come out zero (zero-padded weights+bias)
                        nc.scalar.activation(out=outT[:, h, ci * CHW:(ci + 1) * CHW],
                                             in_=ps[:],
                                             func=mybir.ActivationFunctionType.Identity,
                                             bias=b_sb[:, h:h + 1], scale=1.0)

            for gi in range(ngroups):
                t0 = gi * GT
                gidx = n * ngroups + gi
                if stages < 2:
                    continue

                # logitsT: one K=32 matmul per head over the whole 128-col
                # group -> [128(w at 32-stride), h, 128(v at 32-stride)].
                # Off-diagonal (cross-t) products land in the tile but are
                # never read.
                gc = t0 * 32
                e_bd = ebd_tiles[gidx % 2]
                af_bd = afbd_tiles[gidx % 2]
                if not use_adapt:
                    af_bd = abd_const
                ps_l = ps_a.tile([128, H, 128], mybir.dt.float32, tag="lg")
                if use_adapt:
                    for h in range(H):
                        nc.tensor.matmul(
                            ps_l[:, h, :],
                            psiT_sb[:, h, gc:gc + 128],
                            phiT_sb[:, h, gc:gc + 128],
                            start=True, stop=True)
                if use_adapt:
                    for tl in range(GT):
                        blk = slice(32 * tl, 32 * tl + V)
                        nc.scalar.activation(
                            out=e_bd[blk, :, blk],
                            in_=ps_l[blk, :, blk],
                            func=mybir.ActivationFunctionType.Exp)
                    # softmax denominators, replicated across each 25-row block
                    ps_s = ps_b.tile([128, H * 128], mybir.dt.float32, tag="sm")
                    nc.tensor.matmul(ps_s[:], bdiag_sb[:],
                                     e_bd[:, :, :].rearrange("w h v -> w (h v)"),
                                     start=True, stop=True)
                    ps_s3 = ps_s[:, :].rearrange("w (h v) -> w h v", v=128)
                    r_sb = gpool.tile([128, H, 128], mybir.dt.float32, tag="recip")
                    abase3 = abaseT_sb[:, :].rearrange("w (h v) -> w h v", v=V)
                    for tl in range(GT):
                        blk = slice(32 * tl, 32 * tl + V)
                        nc.vector.reciprocal(r_sb[blk, :, blk], ps_s3[blk, :, blk])
                        nc.vector.tensor_scalar_mul(r_sb[blk, :, blk],
                                                    r_sb[blk, :, blk], lam_sb[blk, :])
                        nc.vector.tensor_mul(r_sb[blk, :, blk], r_sb[blk, :, blk],
                                             e_bd[blk, :, blk])
                        nc.vector.tensor_add(af_bd[blk, :, blk], r_sb[blk, :, blk],
                                             abase3[blk, :, :])
                if stages < 3:
                    continue

                # Y_h = X-group^T @ wdT_h : [128, O] f32 -> SBUF bf16
                ps_yt = ps_y.tile([128, H, O], mybir.dt.float32, tag="y")
                for h in range(H):
                    for k in range(2):
                        nc.tensor.matmul(ps_yt[:, h, :],
                                         x_sb[:, k, t0:t0 + GT, :],
                                         wdT_sb[:, k, h, :],
                                         start=(k == 0), stop=(k == 1))
                y_sb = gpool.tile([128, H, O], bf16, tag="ysb")
                nc.vector.tensor_copy(y_sb[:], ps_yt[:])

                # gate logits
                ps_gt = ps_g.tile([128, O], mybir.dt.float32, tag="gate")
                for k in range(2):
                    nc.tensor.matmul(ps_gt[:], x_sb[:, k, t0:t0 + GT, :],
                                     wgT_sb[:, k, :], start=(k == 0), stop=(k == 1))
                nc.vector.tensor_add(ps_gt[:], ps_gt[:], bg_sb[:])
                gate_sb = gpool.tile([128, O], mybir.dt.float32, tag="gatesb")
                nc.scalar.activation(out=gate_sb[:], in_=ps_gt[:],
                                     func=mybir.ActivationFunctionType.Sigmoid)
                if stages < 4:
                    continue

                # z = sum_h A_finT_bd_h^T @ Y_h : full K=128 matmuls, the
                # block-diagonal af_bd keeps cross-t terms zero.
                ps_zt = ps_z.tile([128, O], mybir.dt.float32, tag="z")
                for h in range(H):
                    nc.tensor.matmul(ps_zt[:], af_bd[:, h, :], y_sb[:, h, :],
                                     start=(h == 0), stop=(h == H - 1))

                # delta = gate * (z + bnT)  (pad rows carry garbage, never read)
                tmp_sb = gpool.tile([128, O], mybir.dt.float32, tag="tmpz")
                nc.vector.scalar_tensor_tensor(
                    out=tmp_sb[:], in0=ps_zt[:], scalar=1.0,
                    in1=bnT_sb[:], op0=mybir.AluOpType.mult,
                    op1=mybir.AluOpType.add)
                dslice = delta_sb[:, gidx, :]
                nc.vector.tensor_mul(dslice, tmp_sb[:], gate_sb[:])
                red_sb = gpool.tile([128, 1], mybir.dt.float32, tag="red")
                for tl in range(GT):
                    blk = slice(32 * tl, 32 * tl + V)
                    nc.vector.tensor_reduce(red_sb[blk, :], delta_sb[blk, gidx, :],
                                            axis=mybir.AxisListType.X,
                                            op=mybir.AluOpType.max,
                                            apply_absolute_value=True)
                    nc.vector.tensor_max(stats_sb[blk, :], stats_sb[blk, :],
                                         red_sb[blk, :])

        # ---- global absmax -> inv scale ----
        if stages < 5:
            nc.vector.memset(stats_sb[:], 1.0)
        allred_sb = dpool.tile([128, 1], mybir.dt.float32)
        nc.gpsimd.partition_all_reduce(allred_sb[:], stats_sb[:], channels=128,
                                       reduce_op=bass_isa.ReduceOp.max)
        nc.sync.dma_start(out=dmax[:], in_=allred_sb[0:1, 0:1])
        inv_sb = dpool.tile([128, 1], mybir.dt.float32)
        nc.vector.reciprocal(inv_sb[:], allred_sb[:])
        nc.vector.tensor_scalar_mul(inv_sb[:], inv_sb[:], 126.0)

        # ---- quantize: transpose [128,128] -> [128,128], uint8 biased ----
        for n in range(nb):
            q_sb = qpool.tile([128, 2, TV], u8, tag="q")
            for gi in range(ngroups):
                t0 = gi * GT
                gidx = n * ngroups + gi
                for half in range(2):
                    ps_t = ps_b.tile([128, 128], bf16, tag="sm")
                    nc.tensor.transpose(
                        ps_t[:],
                        delta_sb[:, gidx, half * 128:(half + 1) * 128],
                        ident_sb[:])
                    # HW's f32->uint8 output conversion rounds to nearest
                    # (CoreSim truncates -- known divergence; HW is truth).
                    for tl in range(GT):
                        nc.vector.tensor_scalar(
                            out=q_sb[:, half, (t0 + tl) * V:(t0 + tl + 1) * V],
                            in0=ps_t[:, 32 * tl:32 * tl + V],
                            scalar1=inv_sb[:], scalar2=128.0,
                            op0=mybir.AluOpType.mult, op1=mybir.AluOpType.add)
            for half in range(2):
                nc.sync.dma_start(out=dq[n, half * 128:(half + 1) * 128, :],
                                  in_=q_sb[:, half, :])

    nc.compile()
    return nc


# ---------------------------------------------------------------------------
# Host-side weight folding
# ---------------------------------------------------------------------------

def fold_weights(inp):
    import ml_dtypes
    bf = ml_dtypes.bfloat16
    s = np.float32(D ** -0.5)
    bn_s = (inp["bn_gamma"] / np.sqrt(inp["bn_var"] + BN_EPS)).astype(np.float32)
    bn_t = (inp["bn_beta"] - inp["bn_mean"] * bn_s).astype(np.float32)
    w = {}
    # [H, C, D]: wphiT[h, c, d] = (W_phi * s)[h*D+d, c]
    w["wphiT"] = np.ascontiguousarray(
        (inp["W_phi"] * s).reshape(H, D, C).transpose(0, 2, 1)).astype(bf)
    w["wpsiT"] = np.ascontiguousarray(
        inp["W_psi"].reshape(H, D, C).transpose(0, 2, 1)).astype(bf)
    w["wgT"] = np.ascontiguousarray(inp["W_g"].T).astype(bf)
    # wdT[h,c,o] = W_d[h,o,c] * bn_s[o]
    w["wdT"] = np.ascontiguousarray(
        (inp["W_d"] * bn_s[None, :, None]).transpose(0, 2, 1)).astype(bf)
    w["bphi"] = np.ascontiguousarray(
        (inp["b_phi"] * s).astype(np.float32).reshape(H, D).T)
    w["bpsi"] = np.ascontiguousarray(
        inp["b_psi"].astype(np.float32).reshape(H, D).T)
    w["bg"] = inp["b_g"].astype(np.float32).reshape(1, O)
    w["bnT"] = (inp["b_d"].sum(axis=0) * bn_s + bn_t).astype(np.float32).reshape(1, O)
    a_base = inp["A_prior"] + np.float32(inp["beta"]) * inp["A_2hop"]  # [H,V,V]
    # abaseT[w, h*V+v] = a_base[h, v, w]
    w["abaseT"] = np.ascontiguousarray(
        a_base.transpose(2, 0, 1).reshape(V, H * V)).astype(np.float32)
    w["lam"] = np.clip(np.float32(inp["lam"]), 0.0, 1.0).reshape(1, 1).astype(np.float32)
    return w


# ---------------------------------------------------------------------------
# SPMD runner (cached jit through bass2jax under axon)
# ---------------------------------------------------------------------------

class SpmdRunner:
    def __init__(self, nc):
        import jax
        import jax.numpy as jnp
        from jax.sharding import Mesh, PartitionSpec as P, NamedSharding
        from jax.experimental.shard_map import shard_map
        import concourse.mybir as mybir
        from concourse import bass2jax

        bass2jax.install_neuronx_cc_hook()
        self.nc = nc
        partition_name = nc.partition_id_tensor.name if nc.partition_id_tensor else None
        in_names, out_names, out_avals = [], [], []
        for alloc in nc.m.functions[0].allocations:
            if not isinstance(alloc, mybir.MemoryLocationSet):
                continue
            name = alloc.memorylocations[0].name
            if alloc.kind == "ExternalInput":
                if name != partition_name:
                    in_names.append(name)
            elif alloc.kind == "ExternalOutput":
                out_names.append(name)
                out_avals.append(jax.core.ShapedArray(
                    tuple(alloc.tensor_shape), mybir.dt.np(alloc.dtype)))
        self.in_names = in_names
        self.out_names = out_names
        self.out_avals = out_avals
        n_params, n_outs = len(in_names), len(out_names)
        bind_in_names = list(in_names) + list(out_names)
        if partition_name is not None:
            bind_in_names.append(partition_name)
        bind_in_names = tuple(bind_in_names)

        def _body(*args):
            operands = list(args)
            if partition_name is not None:
                operands.append(bass2jax.partition_id_tensor())
            outs = bass2jax._bass_exec_p.bind(
                *operands,
                out_avals=tuple(out_avals),
                in_names=bind_in_names,
                out_names=tuple(out_names),
                lowering_input_output_aliases=(),
                sim_require_finite=True,
                sim_require_nnan=True,
                nc=nc,
            )
            return tuple(outs)

        devices = jax.devices()[:NCORES]
        self.mesh = Mesh(np.asarray(devices), ("core",))
        self.sharding = NamedSharding(self.mesh, P("core"))
        in_specs = (P("core"),) * (n_params + n_outs)
        out_specs = (P("core"),) * n_outs
        self.fn = jax.jit(
            shard_map(_body, mesh=self.mesh, in_specs=in_specs,
                      out_specs=out_specs, check_rep=False),
            keep_unused=True,
        )
        # persistent (non-donated) zero-filled output operands, device-resident
        self.zero_bufs = [
            jax.device_put(
                np.zeros((NCORES * a.shape[0], *a.shape[1:]), a.dtype), self.sharding)
            for a in out_avals
        ]
        self._jax = jax

    def __call__(self, global_inputs):
        args = [global_inputs[n] for n in self.in_names]
        outs = self.fn(*args, *self.zero_bufs)
        return dict(zip(self.out_names, outs))


# ---------------------------------------------------------------------------
# Public kernel
# ---------------------------------------------------------------------------

def _kernel_device(inputs):
    import jax
    import ml_dtypes

    if "runner" not in _CACHE:
        _CACHE["runner"] = SpmdRunner(build_nc())
    runner = _CACHE["runner"]

    x = np.asarray(inputs["x"], np.float32)
    x_bf = x.reshape(N, C, T * V).astype(ml_dtypes.bfloat16)
    # kick off the (dominant) x upload before any other host work
    x_dev = jax.device_put(x_bf, runner.sharding)
    w = fold_weights({k: np.asarray(v, np.float32) for k, v in inputs.items()
                      if k != "x"})

    wnames = ("wphiT", "wpsiT", "wgT", "wdT", "bphi", "bpsi", "bg", "bnT",
              "abaseT", "lam")
    cached = _CACHE.get("wdev")
    if cached is None or not all(
            np.array_equal(cached[0][n], w[n]) for n in wnames):
        # stack per-core copies and park them on the devices; weights are
        # tiny but re-uploading ~5MB each call costs ~70ms through the tunnel
        wdev = {}
        for name in wnames:
            arr = w[name]
            stacked = np.broadcast_to(
                arr[None], (NCORES, *arr.shape)).reshape(
                    NCORES * arr.shape[0], *arr.shape[1:])
            wdev[name] = jax.device_put(np.ascontiguousarray(stacked),
                                        runner.sharding)
        _CACHE["wdev"] = (w, wdev)
        cached = _CACHE["wdev"]

    gi = {"x_bf": x_dev}
    gi.update(cached[1])
    outs = runner(gi)
    dq_dev, dmax_dev = outs["dq"], outs["dmax"]

    # overlap D2H of the scale + all dq shards with dequantization
    res = np.empty((N, C, T * V), np.float32)
    shards = sorted(dq_dev.addressable_shards, key=lambda s: s.index[0].start or 0)
    fetched = [None] * NCORES
    dmax_box = [None]

    def fetch_dmax():
        dmax_box[0] = np.asarray(dmax_dev).reshape(NCORES)

    def fetch(i):
        fetched[i] = np.asarray(shards[i].data)

    th_dmax = threading.Thread(target=fetch_dmax)
    th_dmax.start()
    threads = []
    for i in range(NCORES):
        th = threading.Thread(target=fetch, args=(i,))
        th.start()
        threads.append(th)
    th_dmax.join()
    scales = (dmax_box[0] / np.float32(126.0)).astype(np.float32)
    xr = x.reshape(N, C, T * V)
    for i in range(NCORES):
        threads[i].join()
        blk = fetched[i]  # [NB, O, TV] uint8
        s = scales[i]
        sl = slice(i * NB, (i + 1) * NB)
        r = blk.astype(np.float32)
        r -= np.float32(128.0)
        r *= s
        r += xr[sl]
        np.maximum(r, 0.0, out=r)
        res[sl] = r
    return res.reshape(N, C, T, V)


# ---------------------------------------------------------------------------
# Pure-numpy fallback (reference math)
# ---------------------------------------------------------------------------

def _forward_np(x, A_prior, A_2hop, beta, lam, W_phi, b_phi, W_psi, b_psi,
                W_d, b_d, bn_gamma, bn_beta, bn_mean, bn_var, W_g, b_g):
    n, c, t, v = x.shape
    h, d = H, D
    scale = d ** -0.5

    def conv1x1_heads(W, b):
        y = np.einsum('nctv,ec->netv', x, W) + b[None, :, None, None]
        return (y.reshape(n, h, d, t, v).transpose(0, 3, 1, 4, 2)
                 .reshape(n * t, h, v, d))

    phi = conv1x1_heads(W_phi, b_phi)
    psi = conv1x1_heads(W_psi, b_psi)
    logits = np.einsum('bhvd,bhwd->bhvw', phi, psi) * scale
    m = logits.max(axis=-1, keepdims=True)
    e = np.exp(logits - m)
    A_adapt = e / e.sum(axis=-1, keepdims=True)
    lam_c = np.clip(lam, 0.0, 1.0)
    A_final = (A_prior + beta * A_2hop)[None] + lam_c * A_adapt
    feat = x.transpose(0, 2, 3, 1).reshape(n * t, v, c)
    z = np.einsum('bhvw,bwc->bhvc', A_final, feat)
    out = np.einsum('bhvc,hoc->bvo', z, W_d) + b_d.sum(axis=0)
    out = out.reshape(n, t, v, -1).transpose(0, 3, 1, 2)
    inv = 1.0 / np.sqrt(bn_var + BN_EPS)
    out = ((out - bn_mean[None, :, None, None]) * (inv * bn_gamma)[None, :, None, None]
           + bn_beta[None, :, None, None])
    gate = 1.0 / (1.0 + np.exp(-(np.einsum('nctv,oc->notv', x, W_g)
                                 + b_g[None, :, None, None])))
    out = gate * out + x
    return np.maximum(out, 0.0)


def kernel(**inputs) -> np.ndarray:
    try:
        return _kernel_device(inputs)
    except Exception:
        import traceback
        traceback.print_exc()
        args = [np.asarray(inputs[k], np.float32) for k in
                ["x", "A_prior", "A_2hop", "beta", "lam", "W_phi", "b_phi",
                 "W_psi", "b_psi", "W_d", "b_d", "bn_gamma", "bn_beta",
                 "bn_mean", "bn_var", "W_g", "b_g"]]
        return np.asarray(_forward_np(*args), np.float32)


# revision 53
# speedup vs baseline: 1.3776x; 1.3776x over previous
"""HDGC-style GNN message passing on 8 NeuronCores via a fused Bass/Tile kernel.

Data-parallel over N: each core gets N/8=8 batches plus the full (tiny)
adjacencies and 1x1-conv weights.

Wire format (the axon tunnel is the bottleneck at ~70MB/s each way):
 - x ships as bf16 [64, 256, 64*25]  (52.4 MB)
 - the kernel returns delta = gate * bn(conv_d(A_final @ feat)) quantized to
   biased uint8 (26.2 MB) plus one f32 scale per core; the host applies
   out = relu(delta + x) in f32.

Device kernel math (per core, n=8 batches), with host-side folding:
  W_phi' = W_phi * D^-0.5 (and bias), bn folded into W_d and a bias term,
  A_base = A_prior + beta*A_2hop passed transposed, lam clipped.
  feat = x viewed [C, T*V] per batch; per position-block of 5 t's (125 rows):
    phiT/psiT  [96, pos]  = W'^T-stationary matmuls + bias
    logitsT    [125(w), 75(h,v)] = psiT-block^T @ phiT-block  (per t, h)
    e = exp(logitsT) (no max-sub: |logits| small, f32 exp is safe)
    sums_rep   [125, 75] = blockdiag_ones^T @ e  (per-row softmax denominators,
                replicated across each 25-partition block)
    A_finT     = A_baseT_rep + lam * e * recip(sums_rep)   (bf16)
    Y_h        [125(v...), wait rows=positions, cols=o] = X-block^T @ (W_d'bn)^T
    z          [125(v), 256(o)] = sum_h A_finT_h^T @ Y_h   (PSUM accum)
    gate       = sigmoid(X-block^T @ W_g^T + b_g)
    delta      = gate * (z + bnT)        (bf16, SBUF-resident)
  then: global absmax -> inv scale, PE-transpose delta to [o, pos] and emit
  uint8 = round(delta*inv + 128)  (HW's f32->uint8 convert rounds; biased
  codes stay in [2,254], no wrap).

Hardware gotchas baked into the structure (found the hard way):
  - PE array-packing (tile_position / K<128 / M<128 modes) mixed with
    full-mode matmuls is racy on HW: everything here runs as full 128x128
    matmuls over zero-padded operands (block-diagonal A for the per-t z).
  - Large strided DMAs fan out over a shape-dependent number of HW queues
    and their completion was mis-tracked (nondeterministic corruption):
    x lands with a contiguous DMA and is re-laid-out on VectorE.
"""

import threading
import numpy as np

N, C, T, V, H, O = 64, 256, 64, 25, 3, 256
D = 32
HD = H * D  # 96
BN_EPS = 1e-5
NCORES = 8
NB = N // NCORES  # batches per core

_CACHE = {}


# ---------------------------------------------------------------------------
# Device kernel (Bass/Tile)
# ---------------------------------------------------------------------------

def build_nc(nb=NB, t_dim=T, num_devices=NCORES, stages=5, use_adapt=True):
    from contextlib import ExitStack
    import concourse.bass as bass
    import concourse.mybir as mybir
    from concourse import bacc
    from concourse.tile import TileContext
    from concourse.masks import make_identity
    from concourse import bass_isa

    f32 = mybir.dt.float32
    bf16 = mybir.dt.bfloat16
    u8 = mybir.dt.uint8
    TV = t_dim * V
    # PE matmul partition offsets must be 32-aligned, so positions are packed
    # at 32-stride (25 valid + 7 pad rows per t) in groups of 4 t's = 128 rows.
    GT = 4
    assert t_dim % GT == 0
    ngroups = t_dim // GT
    TV32 = t_dim * 32

    nc = bacc.Bacc("TRN2", target_bir_lowering=False, debug=False,
                   num_devices=num_devices)

    x_bf = nc.dram_tensor("x_bf", [nb, C, TV], bf16, kind="ExternalInput")
    wphiT = nc.dram_tensor("wphiT", [H, C, D], bf16, kind="ExternalInput")
    wpsiT = nc.dram_tensor("wpsiT", [H, C, D], bf16, kind="ExternalInput")
    wgT = nc.dram_tensor("wgT", [C, O], bf16, kind="ExternalInput")
    wdT = nc.dram_tensor("wdT", [H, C, O], bf16, kind="ExternalInput")
    bphi = nc.dram_tensor("bphi", [D, H], f32, kind="ExternalInput")
    bpsi = nc.dram_tensor("bpsi", [D, H], f32, kind="ExternalInput")
    bg = nc.dram_tensor("bg", [1, O], f32, kind="ExternalInput")
    bnT = nc.dram_tensor("bnT", [1, O], f32, kind="ExternalInput")
    abaseT = nc.dram_tensor("abaseT", [V, H * V], f32, kind="ExternalInput")
    lam = nc.dram_tensor("lam", [1, 1], f32, kind="ExternalInput")

    dq = nc.dram_tensor("dq", [nb, O, TV], u8, kind="ExternalOutput")
    dmax = nc.dram_tensor("dmax", [1, 1], f32, kind="ExternalOutput")

    def bcast_rows(src_ap, parts):
        """DRAM [1, F] -> AP replicated across `parts` partitions."""
        return bass.AP(tensor=src_ap.tensor, offset=src_ap.offset,
                       ap=[[0, parts]] + list(src_ap.ap[1:]))

    with TileContext(nc) as tc, ExitStack() as ctx:
        consts = ctx.enter_context(tc.tile_pool(name="consts", bufs=1))
        xpool = ctx.enter_context(tc.tile_pool(name="xpool", bufs=2))
        projpool = ctx.enter_context(tc.tile_pool(name="projpool", bufs=2))
        gpool = ctx.enter_context(tc.tile_pool(name="gpool", bufs=3))
        dpool = ctx.enter_context(tc.tile_pool(name="dpool", bufs=1))
        qpool = ctx.enter_context(tc.tile_pool(name="qpool", bufs=2))
        # PSUM budget: 8 banks of 2KB/partition.
        # ps_a "lg": proj [96,400]f32 & logitsT [125,75]f32 share slots (2 banks)
        # ps_b "sm": sums [125,75]f32 & transpose [128,125]bf16 share (2 banks)
        # ps_y [125,3,256]f32 (2 banks), ps_g (1), ps_z (1)  -> total 8
        ps_a = ctx.enter_context(tc.tile_pool(name="ps_a", bufs=2, space="PSUM"))
        ps_b = ctx.enter_context(tc.tile_pool(name="ps_b", bufs=2, space="PSUM"))
        ps_y = ctx.enter_context(tc.tile_pool(name="ps_y", bufs=1, space="PSUM"))
        ps_g = ctx.enter_context(tc.tile_pool(name="ps_g", bufs=1, space="PSUM"))
        ps_z = ctx.enter_context(tc.tile_pool(name="ps_z", bufs=1, space="PSUM"))

        # ---- constants in SBUF ----
        # phi/psi weights are zero-padded from D=32 to 128 output columns so
        # every matmul runs in full 128x128 PE mode: interleaving tiled modes
        # (K=32 / M=32) with full-mode matmuls races on hardware (mode
        # switches need an array drain) and produced nondeterministic output.
        wphiT_sb = consts.tile([128, 2, H, 128], bf16)
        wpsiT_sb = consts.tile([128, 2, H, 128], bf16)
        nc.vector.memset(wphiT_sb[:], 0.0)
        nc.vector.memset(wpsiT_sb[:], 0.0)
        wgT_sb = consts.tile([128, 2, O], bf16)
        wdT_sb = consts.tile([128, 2, H, O], bf16)
        for k in range(2):
            nc.sync.dma_start(out=wgT_sb[:, k, :], in_=wgT[k * 128:(k + 1) * 128, :])
            for h in range(H):
                nc.sync.dma_start(out=wphiT_sb[:, k, h, 0:D],
                                  in_=wphiT[h, k * 128:(k + 1) * 128, :])
                nc.sync.dma_start(out=wpsiT_sb[:, k, h, 0:D],
                                  in_=wpsiT[h, k * 128:(k + 1) * 128, :])
                nc.sync.dma_start(out=wdT_sb[:, k, h, :],
                                  in_=wdT[h, k * 128:(k + 1) * 128, :])
        bphi_sb = consts.tile([128, H], f32)
        bpsi_sb = consts.tile([128, H], f32)
        nc.vector.memset(bphi_sb[:], 0.0)
        nc.vector.memset(bpsi_sb[:], 0.0)
        nc.sync.dma_start(out=bphi_sb[0:D, :], in_=bphi[:])
        nc.sync.dma_start(out=bpsi_sb[0:D, :], in_=bpsi[:])
        bg_sb = consts.tile([128, O], f32)
        bnT_sb = consts.tile([128, O], f32)
        nc.sync.dma_start(out=bg_sb[:], in_=bcast_rows(bg[:], 128))
        nc.sync.dma_start(out=bnT_sb[:], in_=bcast_rows(bnT[:], 128))
        abaseT_sb = consts.tile([128, H * V], f32)
        nc.vector.memset(abaseT_sb[:], 0.0)
        for j in range(GT):
            nc.sync.dma_start(out=abaseT_sb[32 * j:32 * j + V, :], in_=abaseT[:, :])
        lam_sb = consts.tile([128, 1], f32)
        nc.sync.dma_start(out=lam_sb[:], in_=bcast_rows(lam[:], 128))
        # block-diagonal ones (4 blocks of 25x25 at 32-stride)
        bdiag_sb = consts.tile([128, 128], bf16)
        nc.vector.memset(bdiag_sb[:], 0.0)
        for j in range(GT):
            nc.vector.memset(bdiag_sb[32 * j:32 * j + V, 32 * j:32 * j + V], 1.0)
        ident_sb = consts.tile([128, 128], bf16)
        make_identity(nc, ident_sb[:])

        # delta (bf16) for the whole shard + per-partition absmax stats
        delta_sb = dpool.tile([128, nb * ngroups, O], bf16)
        stats_sb = dpool.tile([128, 1], f32)
        nc.vector.memset(stats_sb[:], 0.0)

        # persistent block-diagonal e / A_finT tiles (double-buffered by group
        # parity); off-diagonal and pad entries stay zero forever, so the z
        # matmuls can run as single full-K matmuls per head with no PE tiling.
        ebd_tiles = []
        afbd_tiles = []
        for i in range(2):
            t_e = dpool.tile([128, H, 128], bf16, tag=f"ebd{i}")
            t_a = dpool.tile([128, H, 128], bf16, tag=f"afbd{i}")
            nc.vector.memset(t_e[:], 0.0)
            nc.vector.memset(t_a[:], 0.0)
            ebd_tiles.append(t_e)
            afbd_tiles.append(t_a)
        if not use_adapt:
            # debug: constant block-diagonal A (= A_baseT), no adaptive part
            abd_const = dpool.tile([128, H, 128], bf16, tag="abdc")
            nc.vector.memset(abd_const[:], 0.0)
            for tl in range(GT):
                blk = slice(32 * tl, 32 * tl + V)
                nc.vector.tensor_copy(
                    abd_const[blk, :, blk],
                    abaseT_sb[blk, :].rearrange("w (h v) -> w h v", v=V))

        CHW = min(512, TV32)
        assert TV32 % CHW == 0
        NCHUNK = TV32 // CHW

        for n in range(nb):
            # x, column-padded to 32 per t: [128, k, t, 32] (cols 25..31 zero).
            # DMA lands contiguous; the pad-strided relayout runs on DVE
            # (a strided DMA here fans out across HW queues and raced).
            x_raw = xpool.tile([128, 2, TV], bf16, tag="xraw")
            for k in range(2):
                nc.sync.dma_start(out=x_raw[:, k, :],
                                  in_=x_bf[n, k * 128:(k + 1) * 128, :])
            x_sb = xpool.tile([128, 2, t_dim, 32], bf16, tag="x")
            for k in range(2):
                nc.vector.tensor_copy(
                    x_sb[:, k, :, 0:V],
                    x_raw[:, k, :].rearrange("c (t v) -> c t v", v=V))
                nc.vector.memset(x_sb[:, k, :, V:32], 0.0)

            # phi/psi projections per head: [128(d, zero above 32), H, TV32]
            # bf16 (+bias); d^-0.5 folded into W. Rows 32-127 are zero so the
            # logits matmuls can contract over the full K=128 (full PE mode).
            phiT_sb = projpool.tile([128, H, TV32], bf16, tag="phi")
            psiT_sb = projpool.tile([128, H, TV32], bf16, tag="psi")
            xflat = x_sb[:, :, :, :].rearrange("c k t v -> c k (t v)")
            for (wT_sb, b_sb, outT) in ((wphiT_sb, bphi_sb, phiT_sb),
                                        (wpsiT_sb, bpsi_sb, psiT_sb)):
                for h in range(H):
                    for ci in range(NCHUNK):
                        ps = ps_a.tile([128, CHW], mybir.dt.float32, tag="lg")
                        for k in range(2):
                            nc.tensor.matmul(ps[:], wT_sb[:, k, h, :],
                                             xflat[:, k, ci * CHW:(ci + 1) * CHW],
                                             start=(k == 0), stop=(k == 1))
                        # rows D..127 come out zero (zero-padded weights+bias)
                        nc.scalar.activation(out=outT[:, h, ci * CHW:(ci + 1) * CHW],
                                             in_=ps[:],
                                             func=mybir.ActivationFunctionType.Identity,
                                             bias=b_sb[:, h:h + 1], scale=1.0)

            for gi in range(ngroups):
                t0 = gi * GT
                gidx = n * ngroups + gi
                if stages < 2:
                    continue

                # logitsT: one K=32 matmul per head over the whole 128-col
                # group -> [128(w at 32-stride), h, 128(v at 32-stride)].
                # Off-diagonal (cross-t) products land in the tile but are
                # never read.
                gc = t0 * 32
                e_bd = ebd_tiles[gidx % 2]
                af_bd = afbd_tiles[gidx % 2]
                if not use_adapt:
                    af_bd = abd_const
                ps_l = ps_a.tile([128, H, 128], mybir.dt.float32, tag="lg")
                if use_adapt:
                    for h in range(H):
                        nc.tensor.matmul(
                            ps_l[:, h, :],
                            psiT_sb[:, h, gc:gc + 128],
                            phiT_sb[:, h, gc:gc + 128],
                            start=True, stop=True)
                if use_adapt:
                    for tl in range(GT):
                        blk = slice(32 * tl, 32 * tl + V)
                        nc.scalar.activation(
                            out=e_bd[blk, :, blk],
                            in_=ps_l[blk, :, blk],
                            func=mybir.ActivationFunctionType.Exp)
                    # softmax denominators, replicated across each 25-row block
                    ps_s = ps_b.tile([128, H * 128], mybir.dt.float32, tag="sm")
                    nc.tensor.matmul(ps_s[:], bdiag_sb[:],
                                     e_bd[:, :, :].rearrange("w h v -> w (h v)"),
                                     start=True, stop=True)
                    ps_s3 = ps_s[:, :].rearrange("w (h v) -> w h v", v=128)
                    r_sb = gpool.tile([128, H, 128], mybir.dt.float32, tag="recip")
                    abase3 = abaseT_sb[:, :].rearrange("w (h v) -> w h v", v=V)
                    for tl in range(GT):
                        blk = slice(32 * tl, 32 * tl + V)
                        nc.vector.reciprocal(r_sb[blk, :, blk], ps_s3[blk, :, blk])
                        nc.vector.tensor_scalar_mul(r_sb[blk, :, blk],
                                                    r_sb[blk, :, blk], lam_sb[blk, :])
                        nc.vector.tensor_mul(r_sb[blk, :, blk], r_sb[blk, :, blk],
                                             e_bd[blk, :, blk])
                        nc.vector.tensor_add(af_bd[blk, :, blk], r_sb[blk, :, blk],
                                             abase3[blk, :, :])
                if stages < 3:
                    continue

                # Y_h = X-group^T @ wdT_h : [128, O] f32 -> SBUF bf16
                ps_yt = ps_y.tile([128, H, O], mybir.dt.float32, tag="y")
                for h in range(H):
                    for k in range(2):
                        nc.tensor.matmul(ps_yt[:, h, :],
                                         x_sb[:, k, t0:t0 + GT, :],
                                         wdT_sb[:, k, h, :],
                                         start=(k == 0), stop=(k == 1))
                y_sb = gpool.tile([128, H, O], bf16, tag="ysb")
                nc.vector.tensor_copy(y_sb[:], ps_yt[:])

                # gate logits
                ps_gt = ps_g.tile([128, O], mybir.dt.float32, tag="gate")
                for k in range(2):
                    nc.tensor.matmul(ps_gt[:], x_sb[:, k, t0:t0 + GT, :],
                                     wgT_sb[:, k, :], start=(k == 0), stop=(k == 1))
                nc.vector.tensor_add(ps_gt[:], ps_gt[:], bg_sb[:])
                gate_sb = gpool.tile([128, O], mybir.dt.float32, tag="gatesb")
                nc.scalar.activation(out=gate_sb[:], in_=ps_gt[:],
                                     func=mybir.ActivationFunctionType.Sigmoid)
                if stages < 4:
                    continue

                # z = sum_h A_finT_bd_h^T @ Y_h : full K=128 matmuls, the
                # block-diagonal af_bd keeps cross-t terms zero.
                ps_zt = ps_z.tile([128, O], mybir.dt.float32, tag="z")
                for h in range(H):
                    nc.tensor.matmul(ps_zt[:], af_bd[:, h, :], y_sb[:, h, :],
                                     start=(h == 0), stop=(h == H - 1))

                # delta = gate * (z + bnT)  (pad rows carry garbage, never read)
                tmp_sb = gpool.tile([128, O], mybir.dt.float32, tag="tmpz")
                nc.vector.scalar_tensor_tensor(
                    out=tmp_sb[:], in0=ps_zt[:], scalar=1.0,
                    in1=bnT_sb[:], op0=mybir.AluOpType.mult,
                    op1=mybir.AluOpType.add)
                dslice = delta_sb[:, gidx, :]
                nc.vector.tensor_mul(dslice, tmp_sb[:], gate_sb[:])
                red_sb = gpool.tile([128, 1], mybir.dt.float32, tag="red")
                for tl in range(GT):
                    blk = slice(32 * tl, 32 * tl + V)
                    nc.vector.tensor_reduce(red_sb[blk, :], delta_sb[blk, gidx, :],
                                            axis=mybir.AxisListType.X,
                                            op=mybir.AluOpType.max,
                                            apply_absolute_value=True)
                    nc.vector.tensor_max(stats_sb[blk, :], stats_sb[blk, :],
                                         red_sb[blk, :])

        # ---- global absmax -> inv scale ----
        if stages < 5:
            nc.vector.memset(stats_sb[:], 1.0)
        allred_sb = dpool.tile([128, 1], mybir.dt.float32)
        nc.gpsimd.partition_all_reduce(allred_sb[:], stats_sb[:], channels=128,
                                       reduce_op=bass_isa.ReduceOp.max)
        nc.sync.dma_start(out=dmax[:], in_=allred_sb[0:1, 0:1])
        inv_sb = dpool.tile([128, 1], mybir.dt.float32)
        nc.vector.reciprocal(inv_sb[:], allred_sb[:])
        nc.vector.tensor_scalar_mul(inv_sb[:], inv_sb[:], 126.0)

        # ---- quantize: transpose [128,128] -> [128,128], uint8 biased ----
        for n in range(nb):
            q_sb = qpool.tile([128, 2, TV], u8, tag="q")
            for gi in range(ngroups):
                t0 = gi * GT
                gidx = n * ngroups + gi
                for half in range(2):
                    ps_t = ps_b.tile([128, 128], bf16, tag="sm")
                    nc.tensor.transpose(
                        ps_t[:],
                        delta_sb[:, gidx, half * 128:(half + 1) * 128],
                        ident_sb[:])
                    # HW's f32->uint8 output conversion rounds to nearest
                    # (CoreSim truncates -- known divergence; HW is truth).
                    for tl in range(GT):
                        nc.vector.tensor_scalar(
                            out=q_sb[:, half, (t0 + tl) * V:(t0 + tl + 1) * V],
                            in0=ps_t[:, 32 * tl:32 * tl + V],
                            scalar1=inv_sb[:], scalar2=128.0,
                            op0=mybir.AluOpType.mult, op1=mybir.AluOpType.add)
            for half in range(2):
                nc.sync.dma_start(out=dq[n, half * 128:(half + 1) * 128, :],
                                  in_=q_sb[:, half, :])

    nc.compile()
    return nc


# ---------------------------------------------------------------------------
# Host-side weight folding
# ---------------------------------------------------------------------------

def fold_weights(inp):
    import ml_dtypes
    bf = ml_dtypes.bfloat16
    s = np.float32(D ** -0.5)
    bn_s = (inp["bn_gamma"] / np.sqrt(inp["bn_var"] + BN_EPS)).astype(np.float32)
    bn_t = (inp["bn_beta"] - inp["bn_mean"] * bn_s).astype(np.float32)
    w = {}
    # [H, C, D]: wphiT[h, c, d] = (W_phi * s)[h*D+d, c]
    w["wphiT"] = np.ascontiguousarray(
        (inp["W_phi"] * s).reshape(H, D, C).transpose(0, 2, 1)).astype(bf)
    w["wpsiT"] = np.ascontiguousarray(
        inp["W_psi"].reshape(H, D, C).transpose(0, 2, 1)).astype(bf)
    w["wgT"] = np.ascontiguousarray(inp["W_g"].T).astype(bf)
    # wdT[h,c,o] = W_d[h,o,c] * bn_s[o]
    w["wdT"] = np.ascontiguousarray(
        (inp["W_d"] * bn_s[None, :, None]).transpose(0, 2, 1)).astype(bf)
    w["bphi"] = np.ascontiguousarray(
        (inp["b_phi"] * s).astype(np.float32).reshape(H, D).T)
    w["bpsi"] = np.ascontiguousarray(
        inp["b_psi"].astype(np.float32).reshape(H, D).T)
    w["bg"] = inp["b_g"].astype(np.float32).reshape(1, O)
    w["bnT"] = (inp["b_d"].sum(axis=0) * bn_s + bn_t).astype(np.float32).reshape(1, O)
    a_base = inp["A_prior"] + np.float32(inp["beta"]) * inp["A_2hop"]  # [H,V,V]
    # abaseT[w, h*V+v] = a_base[h, v, w]
    w["abaseT"] = np.ascontiguousarray(
        a_base.transpose(2, 0, 1).reshape(V, H * V)).astype(np.float32)
    w["lam"] = np.clip(np.float32(inp["lam"]), 0.0, 1.0).reshape(1, 1).astype(np.float32)
    return w


# ---------------------------------------------------------------------------
# SPMD runner (cached jit through bass2jax under axon)
# ---------------------------------------------------------------------------

class SpmdRunner:
    def __init__(self, nc):
        import jax
        import jax.numpy as jnp
        from jax.sharding import Mesh, PartitionSpec as P, NamedSharding
        from jax.experimental.shard_map import shard_map
        import concourse.mybir as mybir
        from concourse import bass2jax

        bass2jax.install_neuronx_cc_hook()
        self.nc = nc
        partition_name = nc.partition_id_tensor.name if nc.partition_id_tensor else None
        in_names, out_names, out_avals = [], [], []
        for alloc in nc.m.functions[0].allocations:
            if not isinstance(alloc, mybir.MemoryLocationSet):
                continue
            name = alloc.memorylocations[0].name
            if alloc.kind == "ExternalInput":
                if name != partition_name:
                    in_names.append(name)
            elif alloc.kind == "ExternalOutput":
                out_names.append(name)
                out_avals.append(jax.core.ShapedArray(
                    tuple(alloc.tensor_shape), mybir.dt.np(alloc.dtype)))
        self.in_names = in_names
        self.out_names = out_names
        self.out_avals = out_avals
        n_params, n_outs = len(in_names), len(out_names)
        bind_in_names = list(in_names) + list(out_names)
        if partition_name is not None:
            bind_in_names.append(partition_name)
        bind_in_names = tuple(bind_in_names)

        def _body(*args):
            operands = list(args)
            if partition_name is not None:
                operands.append(bass2jax.partition_id_tensor())
            outs = bass2jax._bass_exec_p.bind(
                *operands,
                out_avals=tuple(out_avals),
                in_names=bind_in_names,
                out_names=tuple(out_names),
                lowering_input_output_aliases=(),
                sim_require_finite=True,
                sim_require_nnan=True,
                nc=nc,
            )
            return tuple(outs)

        devices = jax.devices()[:NCORES]
        self.mesh = Mesh(np.asarray(devices), ("core",))
        self.sharding = NamedSharding(self.mesh, P("core"))
        in_specs = (P("core"),) * (n_params + n_outs)
        out_specs = (P("core"),) * n_outs
        self.fn = jax.jit(
            shard_map(_body, mesh=self.mesh, in_specs=in_specs,
                      out_specs=out_specs, check_rep=False),
            keep_unused=True,
        )
        # persistent (non-donated) zero-filled output operands, device-resident
        self.zero_bufs = [
            jax.device_put(
                np.zeros((NCORES * a.shape[0], *a.shape[1:]), a.dtype), self.sharding)
            for a in out_avals
        ]
        self._jax = jax

    def __call__(self, global_inputs):
        args = [global_inputs[n] for n in self.in_names]
        outs = self.fn(*args, *self.zero_bufs)
        return dict(zip(self.out_names, outs))


# ---------------------------------------------------------------------------
# Public kernel
# ---------------------------------------------------------------------------

def _kernel_device(inputs):
    import jax
    import ml_dtypes

    if "runner" not in _CACHE:
        _CACHE["runner"] = SpmdRunner(build_nc())
    runner = _CACHE["runner"]

    x = np.asarray(inputs["x"], np.float32)
    x_bf = x.reshape(N, C, T * V).astype(ml_dtypes.bfloat16)
    # kick off the (dominant) x upload before any other host work
    x_dev = jax.device_put(x_bf, runner.sharding)
    w = fold_weights({k: np.asarray(v, np.float32) for k, v in inputs.items()
                      if k != "x"})

    wnames = ("wphiT", "wpsiT", "wgT", "wdT", "bphi", "bpsi", "bg", "bnT",
              "abaseT", "lam")
    cached = _CACHE.get("wdev")
    if cached is None or not all(
            np.array_equal(cached[0][n], w[n]) for n in wnames):
        # stack per-core copies and park them on the devices; weights are
        # tiny but re-uploading ~5MB each call costs ~70ms through the tunnel
        wdev = {}
        for name in wnames:
            arr = w[name]
            stacked = np.broadcast_to(
                arr[None], (NCORES, *arr.shape)).reshape(
                    NCORES * arr.shape[0], *arr.shape[1:])
            wdev[name] = jax.device_put(np.ascontiguousarray(stacked),
                                        runner.sharding)
        _CACHE["wdev"] = (w, wdev)
        cached = _CACHE["wdev"]

    gi = {"x_bf": x_dev}
    gi.update(cached[1])
    outs = runner(gi)
    dq_dev, dmax_dev = outs["dq"], outs["dmax"]

    # overlap D2H of the scale + all dq shards with dequantization
    res = np.empty((N, C, T * V), np.float32)
    shards = sorted(dq_dev.addressable_shards, key=lambda s: s.index[0].start or 0)
    fetched = [None] * NCORES
    dmax_box = [None]

    def fetch_dmax():
        dmax_box[0] = np.asarray(dmax_dev).reshape(NCORES)

    def fetch(i):
        fetched[i] = np.asarray(shards[i].data)

    th_dmax = threading.Thread(target=fetch_dmax)
    th_dmax.start()
    threads = []
    for i in range(NCORES):
        th = threading.Thread(target=fetch, args=(i,))
        th.start()
        threads.append(th)
    th_dmax.join()
    scales = (dmax_box[0] / np.float32(126.0)).astype(np.float32)
    xr = x.reshape(N, C, T * V)
    for i in range(NCORES):
        threads[i].join()
        blk = fetched[i]  # [NB, O, TV] uint8
        s = scales[i]
        sl = slice(i * NB, (i + 1) * NB)
        r = blk.astype(np.float32)
        r -= np.float32(128.0)
        r *= s
        r += xr[sl]
        np.maximum(r, 0.0, out=r)
        res[sl] = r
    return res.reshape(N, C, T, V)


# ---------------------------------------------------------------------------
# Pure-numpy fallback (reference math)
# ---------------------------------------------------------------------------

def _forward_np(x, A_prior, A_2hop, beta, lam, W_phi, b_phi, W_psi, b_psi,
                W_d, b_d, bn_gamma, bn_beta, bn_mean, bn_var, W_g, b_g):
    n, c, t, v = x.shape
    h, d = H, D
    scale = d ** -0.5

    def conv1x1_heads(W, b):
        y = np.einsum('nctv,ec->netv', x, W) + b[None, :, None, None]
        return (y.reshape(n, h, d, t, v).transpose(0, 3, 1, 4, 2)
                 .reshape(n * t, h, v, d))

    phi = conv1x1_heads(W_phi, b_phi)
    psi = conv1x1_heads(W_psi, b_psi)
    logits = np.einsum('bhvd,bhwd->bhvw', phi, psi) * scale
    m = logits.max(axis=-1, keepdims=True)
    e = np.exp(logits - m)
    A_adapt = e / e.sum(axis=-1, keepdims=True)
    lam_c = np.clip(lam, 0.0, 1.0)
    A_final = (A_prior + beta * A_2hop)[None] + lam_c * A_adapt
    feat = x.transpose(0, 2, 3, 1).reshape(n * t, v, c)
    z = np.einsum('bhvw,bwc->bhvc', A_final, feat)
    out = np.einsum('bhvc,hoc->bvo', z, W_d) + b_d.sum(axis=0)
    out = out.reshape(n, t, v, -1).transpose(0, 3, 1, 2)
    inv = 1.0 / np.sqrt(bn_var + BN_EPS)
    out = ((out - bn_mean[None, :, None, None]) * (inv * bn_gamma)[None, :, None, None]
           + bn_beta[None, :, None, None])
    gate = 1.0 / (1.0 + np.exp(-(np.einsum('nctv,oc->notv', x, W_g)
                                 + b_g[None, :, None, None])))
    out = gate * out + x
    return np.maximum(out, 0.0)


def kernel(**inputs) -> np.ndarray:
    try:
        return _kernel_device(inputs)
    except Exception:
        import traceback
        traceback.print_exc()
        args = [np.asarray(inputs[k], np.float32) for k in
                ["x", "A_prior", "A_2hop", "beta", "lam", "W_phi", "b_phi",
                 "W_psi", "b_psi", "W_d", "b_d", "bn_gamma", "bn_beta",
                 "bn_mean", "bn_var", "W_g", "b_g"]]
        return np.asarray(_forward_np(*args), np.float32)


# revision 54
# speedup vs baseline: 1.5650x; 1.1360x over previous
"""HDGC-style GNN message passing on 8 NeuronCores via a fused Bass/Tile kernel.

Data-parallel over N: each core gets N/8=8 batches plus the full (tiny)
adjacencies and 1x1-conv weights.

Wire format (the axon tunnel is the bottleneck at ~70MB/s each way):
 - x ships as bf16 [64, 256, 64*25]  (52.4 MB)
 - the kernel returns delta = gate * bn(conv_d(A_final @ feat)) quantized to
   biased uint8 (26.2 MB) plus one f32 scale per core; the host applies
   out = relu(delta + x) in f32.

Device kernel math (per core, n=8 batches), with host-side folding:
  W_phi' = W_phi * D^-0.5 (and bias), bn folded into W_d and a bias term,
  A_base = A_prior + beta*A_2hop passed transposed, lam clipped.
  feat = x viewed [C, T*V] per batch; per position-block of 5 t's (125 rows):
    phiT/psiT  [96, pos]  = W'^T-stationary matmuls + bias
    logitsT    [125(w), 75(h,v)] = psiT-block^T @ phiT-block  (per t, h)
    e = exp(logitsT) (no max-sub: |logits| small, f32 exp is safe)
    sums_rep   [125, 75] = blockdiag_ones^T @ e  (per-row softmax denominators,
                replicated across each 25-partition block)
    A_finT     = A_baseT_rep + lam * e * recip(sums_rep)   (bf16)
    Y_h        [125(v...), wait rows=positions, cols=o] = X-block^T @ (W_d'bn)^T
    z          [125(v), 256(o)] = sum_h A_finT_h^T @ Y_h   (PSUM accum)
    gate       = sigmoid(X-block^T @ W_g^T + b_g)
    delta      = gate * (z + bnT)        (bf16, SBUF-resident)
  then: global absmax -> inv scale, PE-transpose delta to [o, pos] and emit
  uint8 = round(delta*inv + 128)  (HW's f32->uint8 convert rounds; biased
  codes stay in [2,254], no wrap).

Hardware gotchas baked into the structure (found the hard way):
  - PE array-packing (tile_position / K<128 / M<128 modes) mixed with
    full-mode matmuls is racy on HW: everything here runs as full 128x128
    matmuls over zero-padded operands (block-diagonal A for the per-t z).
  - Large strided DMAs fan out over a shape-dependent number of HW queues
    and their completion was mis-tracked (nondeterministic corruption):
    x lands with a contiguous DMA and is re-laid-out on VectorE.
"""

import threading
import numpy as np

N, C, T, V, H, O = 64, 256, 64, 25, 3, 256
D = 32
HD = H * D  # 96
BN_EPS = 1e-5
NCORES = 8
NB = N // NCORES  # batches per core

_CACHE = {}


# ---------------------------------------------------------------------------
# Device kernel (Bass/Tile)
# ---------------------------------------------------------------------------

def build_nc(nb=NB, t_dim=T, num_devices=NCORES, stages=5, use_adapt=True):
    from contextlib import ExitStack
    import concourse.bass as bass
    import concourse.mybir as mybir
    from concourse import bacc
    from concourse.tile import TileContext
    from concourse.masks import make_identity
    from concourse import bass_isa

    f32 = mybir.dt.float32
    bf16 = mybir.dt.bfloat16
    u8 = mybir.dt.uint8
    TV = t_dim * V
    # PE matmul partition offsets must be 32-aligned, so positions are packed
    # at 32-stride (25 valid + 7 pad rows per t) in groups of 4 t's = 128 rows.
    GT = 4
    assert t_dim % GT == 0
    ngroups = t_dim // GT
    TV32 = t_dim * 32

    nc = bacc.Bacc("TRN2", target_bir_lowering=False, debug=False,
                   num_devices=num_devices)

    x_bf = nc.dram_tensor("x_bf", [nb, C, TV], bf16, kind="ExternalInput")
    wphiT = nc.dram_tensor("wphiT", [H, C, D], bf16, kind="ExternalInput")
    wpsiT = nc.dram_tensor("wpsiT", [H, C, D], bf16, kind="ExternalInput")
    wgT = nc.dram_tensor("wgT", [C, O], bf16, kind="ExternalInput")
    wdT = nc.dram_tensor("wdT", [H, C, O], bf16, kind="ExternalInput")
    bphi = nc.dram_tensor("bphi", [D, H], f32, kind="ExternalInput")
    bpsi = nc.dram_tensor("bpsi", [D, H], f32, kind="ExternalInput")
    bg = nc.dram_tensor("bg", [1, O], f32, kind="ExternalInput")
    bnT = nc.dram_tensor("bnT", [1, O], f32, kind="ExternalInput")
    abaseT = nc.dram_tensor("abaseT", [V, H * V], f32, kind="ExternalInput")
    lam = nc.dram_tensor("lam", [1, 1], f32, kind="ExternalInput")

    dq = nc.dram_tensor("dq", [nb, O, TV], u8, kind="ExternalOutput")
    dmax = nc.dram_tensor("dmax", [1, 1], f32, kind="ExternalOutput")

    def bcast_rows(src_ap, parts):
        """DRAM [1, F] -> AP replicated across `parts` partitions."""
        return bass.AP(tensor=src_ap.tensor, offset=src_ap.offset,
                       ap=[[0, parts]] + list(src_ap.ap[1:]))

    with TileContext(nc) as tc, ExitStack() as ctx:
        consts = ctx.enter_context(tc.tile_pool(name="consts", bufs=1))
        xpool = ctx.enter_context(tc.tile_pool(name="xpool", bufs=2))
        projpool = ctx.enter_context(tc.tile_pool(name="projpool", bufs=2))
        gpool = ctx.enter_context(tc.tile_pool(name="gpool", bufs=3))
        dpool = ctx.enter_context(tc.tile_pool(name="dpool", bufs=1))
        qpool = ctx.enter_context(tc.tile_pool(name="qpool", bufs=2))
        # PSUM budget: 8 banks of 2KB/partition.
        # ps_a "lg": proj [96,400]f32 & logitsT [125,75]f32 share slots (2 banks)
        # ps_b "sm": sums [125,75]f32 & transpose [128,125]bf16 share (2 banks)
        # ps_y [125,3,256]f32 (2 banks), ps_g (1), ps_z (1)  -> total 8
        ps_a = ctx.enter_context(tc.tile_pool(name="ps_a", bufs=2, space="PSUM"))
        ps_b = ctx.enter_context(tc.tile_pool(name="ps_b", bufs=2, space="PSUM"))
        ps_y = ctx.enter_context(tc.tile_pool(name="ps_y", bufs=1, space="PSUM"))
        ps_g = ctx.enter_context(tc.tile_pool(name="ps_g", bufs=1, space="PSUM"))
        ps_z = ctx.enter_context(tc.tile_pool(name="ps_z", bufs=1, space="PSUM"))

        # ---- constants in SBUF ----
        # phi/psi weights are zero-padded from D=32 to 128 output columns so
        # every matmul runs in full 128x128 PE mode: interleaving tiled modes
        # (K=32 / M=32) with full-mode matmuls races on hardware (mode
        # switches need an array drain) and produced nondeterministic output.
        wphiT_sb = consts.tile([128, 2, H, 128], bf16)
        wpsiT_sb = consts.tile([128, 2, H, 128], bf16)
        nc.vector.memset(wphiT_sb[:], 0.0)
        nc.vector.memset(wpsiT_sb[:], 0.0)
        wgT_sb = consts.tile([128, 2, O], bf16)
        wdT_sb = consts.tile([128, 2, H, O], bf16)
        for k in range(2):
            nc.sync.dma_start(out=wgT_sb[:, k, :], in_=wgT[k * 128:(k + 1) * 128, :])
            for h in range(H):
                nc.sync.dma_start(out=wphiT_sb[:, k, h, 0:D],
                                  in_=wphiT[h, k * 128:(k + 1) * 128, :])
                nc.sync.dma_start(out=wpsiT_sb[:, k, h, 0:D],
                                  in_=wpsiT[h, k * 128:(k + 1) * 128, :])
                nc.sync.dma_start(out=wdT_sb[:, k, h, :],
                                  in_=wdT[h, k * 128:(k + 1) * 128, :])
        bphi_sb = consts.tile([128, H], f32)
        bpsi_sb = consts.tile([128, H], f32)
        nc.vector.memset(bphi_sb[:], 0.0)
        nc.vector.memset(bpsi_sb[:], 0.0)
        nc.sync.dma_start(out=bphi_sb[0:D, :], in_=bphi[:])
        nc.sync.dma_start(out=bpsi_sb[0:D, :], in_=bpsi[:])
        bg_sb = consts.tile([128, O], f32)
        bnT_sb = consts.tile([128, O], f32)
        nc.sync.dma_start(out=bg_sb[:], in_=bcast_rows(bg[:], 128))
        nc.sync.dma_start(out=bnT_sb[:], in_=bcast_rows(bnT[:], 128))
        abaseT_sb = consts.tile([128, H * V], f32)
        nc.vector.memset(abaseT_sb[:], 0.0)
        for j in range(GT):
            nc.sync.dma_start(out=abaseT_sb[32 * j:32 * j + V, :], in_=abaseT[:, :])
        lam_sb = consts.tile([128, 1], f32)
        nc.sync.dma_start(out=lam_sb[:], in_=bcast_rows(lam[:], 128))
        # block-diagonal ones (4 blocks of 25x25 at 32-stride)
        bdiag_sb = consts.tile([128, 128], bf16)
        nc.vector.memset(bdiag_sb[:], 0.0)
        for j in range(GT):
            nc.vector.memset(bdiag_sb[32 * j:32 * j + V, 32 * j:32 * j + V], 1.0)
        ident_sb = consts.tile([128, 128], bf16)
        make_identity(nc, ident_sb[:])

        # delta (bf16) for the whole shard + per-partition absmax stats
        delta_sb = dpool.tile([128, nb * ngroups, O], bf16)
        stats_sb = dpool.tile([128, 1], f32)
        nc.vector.memset(stats_sb[:], 0.0)

        # persistent block-diagonal e / A_finT tiles (double-buffered by group
        # parity); off-diagonal and pad entries stay zero forever, so the z
        # matmuls can run as single full-K matmuls per head with no PE tiling.
        ebd_tiles = []
        afbd_tiles = []
        for i in range(2):
            t_e = dpool.tile([128, H, 128], bf16, tag=f"ebd{i}")
            t_a = dpool.tile([128, H, 128], bf16, tag=f"afbd{i}")
            nc.vector.memset(t_e[:], 0.0)
            nc.vector.memset(t_a[:], 0.0)
            ebd_tiles.append(t_e)
            afbd_tiles.append(t_a)
        if not use_adapt:
            # debug: constant block-diagonal A (= A_baseT), no adaptive part
            abd_const = dpool.tile([128, H, 128], bf16, tag="abdc")
            nc.vector.memset(abd_const[:], 0.0)
            for tl in range(GT):
                blk = slice(32 * tl, 32 * tl + V)
                nc.vector.tensor_copy(
                    abd_const[blk, :, blk],
                    abaseT_sb[blk, :].rearrange("w (h v) -> w h v", v=V))

        CHW = min(512, TV32)
        assert TV32 % CHW == 0
        NCHUNK = TV32 // CHW

        for n in range(nb):
            # x, column-padded to 32 per t: [128, k, t, 32] (cols 25..31 zero).
            # DMA lands contiguous; the pad-strided relayout runs on DVE
            # (a strided DMA here fans out across HW queues and raced).
            x_raw = xpool.tile([128, 2, TV], bf16, tag="xraw")
            for k in range(2):
                nc.sync.dma_start(out=x_raw[:, k, :],
                                  in_=x_bf[n, k * 128:(k + 1) * 128, :])
            x_sb = xpool.tile([128, 2, t_dim, 32], bf16, tag="x")
            for k in range(2):
                nc.vector.tensor_copy(
                    x_sb[:, k, :, 0:V],
                    x_raw[:, k, :].rearrange("c (t v) -> c t v", v=V))
                nc.vector.memset(x_sb[:, k, :, V:32], 0.0)

            # phi/psi projections per head: [128(d, zero above 32), H, TV32]
            # bf16 (+bias); d^-0.5 folded into W. Rows 32-127 are zero so the
            # logits matmuls can contract over the full K=128 (full PE mode).
            phiT_sb = projpool.tile([128, H, TV32], bf16, tag="phi")
            psiT_sb = projpool.tile([128, H, TV32], bf16, tag="psi")
            xflat = x_sb[:, :, :, :].rearrange("c k t v -> c k (t v)")
            for (wT_sb, b_sb, outT) in ((wphiT_sb, bphi_sb, phiT_sb),
                                        (wpsiT_sb, bpsi_sb, psiT_sb)):
                for h in range(H):
                    for ci in range(NCHUNK):
                        ps = ps_a.tile([128, CHW], mybir.dt.float32, tag="lg")
                        for k in range(2):
                            nc.tensor.matmul(ps[:], wT_sb[:, k, h, :],
                                             xflat[:, k, ci * CHW:(ci + 1) * CHW],
                                             start=(k == 0), stop=(k == 1))
                        # rows D..127 come out zero (zero-padded weights+bias)
                        nc.scalar.activation(out=outT[:, h, ci * CHW:(ci + 1) * CHW],
                                             in_=ps[:],
                                             func=mybir.ActivationFunctionType.Identity,
                                             bias=b_sb[:, h:h + 1], scale=1.0)

            for gi in range(ngroups):
                t0 = gi * GT
                gidx = n * ngroups + gi
                if stages < 2:
                    continue

                # logitsT: one K=32 matmul per head over the whole 128-col
                # group -> [128(w at 32-stride), h, 128(v at 32-stride)].
                # Off-diagonal (cross-t) products land in the tile but are
                # never read.
                gc = t0 * 32
                e_bd = ebd_tiles[gidx % 2]
                af_bd = afbd_tiles[gidx % 2]
                if not use_adapt:
                    af_bd = abd_const
                ps_l = ps_a.tile([128, H, 128], mybir.dt.float32, tag="lg")
                if use_adapt:
                    for h in range(H):
                        nc.tensor.matmul(
                            ps_l[:, h, :],
                            psiT_sb[:, h, gc:gc + 128],
                            phiT_sb[:, h, gc:gc + 128],
                            start=True, stop=True)
                if use_adapt:
                    for tl in range(GT):
                        blk = slice(32 * tl, 32 * tl + V)
                        nc.scalar.activation(
                            out=e_bd[blk, :, blk],
                            in_=ps_l[blk, :, blk],
                            func=mybir.ActivationFunctionType.Exp)
                    # softmax denominators, replicated across each 25-row block
                    ps_s = ps_b.tile([128, H * 128], mybir.dt.float32, tag="sm")
                    nc.tensor.matmul(ps_s[:], bdiag_sb[:],
                                     e_bd[:, :, :].rearrange("w h v -> w (h v)"),
                                     start=True, stop=True)
                    ps_s3 = ps_s[:, :].rearrange("w (h v) -> w h v", v=128)
                    r_sb = gpool.tile([128, H, 128], mybir.dt.float32, tag="recip")
                    abase3 = abaseT_sb[:, :].rearrange("w (h v) -> w h v", v=V)
                    for tl in range(GT):
                        blk = slice(32 * tl, 32 * tl + V)
                        nc.vector.reciprocal(r_sb[blk, :, blk], ps_s3[blk, :, blk])
                        nc.vector.tensor_scalar_mul(r_sb[blk, :, blk],
                                                    r_sb[blk, :, blk], lam_sb[blk, :])
                        nc.vector.tensor_mul(r_sb[blk, :, blk], r_sb[blk, :, blk],
                                             e_bd[blk, :, blk])
                        nc.vector.tensor_add(af_bd[blk, :, blk], r_sb[blk, :, blk],
                                             abase3[blk, :, :])
                if stages < 3:
                    continue

                # Y_h = X-group^T @ wdT_h : [128, O] f32 -> SBUF bf16
                ps_yt = ps_y.tile([128, H, O], mybir.dt.float32, tag="y")
                for h in range(H):
                    for k in range(2):
                        nc.tensor.matmul(ps_yt[:, h, :],
                                         x_sb[:, k, t0:t0 + GT, :],
                                         wdT_sb[:, k, h, :],
                                         start=(k == 0), stop=(k == 1))
                y_sb = gpool.tile([128, H, O], bf16, tag="ysb")
                nc.vector.tensor_copy(y_sb[:], ps_yt[:])

                # gate logits
                ps_gt = ps_g.tile([128, O], mybir.dt.float32, tag="gate")
                for k in range(2):
                    nc.tensor.matmul(ps_gt[:], x_sb[:, k, t0:t0 + GT, :],
                                     wgT_sb[:, k, :], start=(k == 0), stop=(k == 1))
                nc.vector.tensor_add(ps_gt[:], ps_gt[:], bg_sb[:])
                gate_sb = gpool.tile([128, O], mybir.dt.float32, tag="gatesb")
                nc.scalar.activation(out=gate_sb[:], in_=ps_gt[:],
                                     func=mybir.ActivationFunctionType.Sigmoid)
                if stages < 4:
                    continue

                # z = sum_h A_finT_bd_h^T @ Y_h : full K=128 matmuls, the
                # block-diagonal af_bd keeps cross-t terms zero.
                ps_zt = ps_z.tile([128, O], mybir.dt.float32, tag="z")
                for h in range(H):
                    nc.tensor.matmul(ps_zt[:], af_bd[:, h, :], y_sb[:, h, :],
                                     start=(h == 0), stop=(h == H - 1))

                # delta = gate * (z + bnT)  (pad rows carry garbage, never read)
                tmp_sb = gpool.tile([128, O], mybir.dt.float32, tag="tmpz")
                nc.vector.scalar_tensor_tensor(
                    out=tmp_sb[:], in0=ps_zt[:], scalar=1.0,
                    in1=bnT_sb[:], op0=mybir.AluOpType.mult,
                    op1=mybir.AluOpType.add)
                dslice = delta_sb[:, gidx, :]
                nc.vector.tensor_mul(dslice, tmp_sb[:], gate_sb[:])
                red_sb = gpool.tile([128, 1], mybir.dt.float32, tag="red")
                for tl in range(GT):
                    blk = slice(32 * tl, 32 * tl + V)
                    nc.vector.tensor_reduce(red_sb[blk, :], delta_sb[blk, gidx, :],
                                            axis=mybir.AxisListType.X,
                                            op=mybir.AluOpType.max,
                                            apply_absolute_value=True)
                    nc.vector.tensor_max(stats_sb[blk, :], stats_sb[blk, :],
                                         red_sb[blk, :])

        # ---- global absmax -> inv scale ----
        if stages < 5:
            nc.vector.memset(stats_sb[:], 1.0)
        allred_sb = dpool.tile([128, 1], mybir.dt.float32)
        nc.gpsimd.partition_all_reduce(allred_sb[:], stats_sb[:], channels=128,
                                       reduce_op=bass_isa.ReduceOp.max)
        nc.sync.dma_start(out=dmax[:], in_=allred_sb[0:1, 0:1])
        inv_sb = dpool.tile([128, 1], mybir.dt.float32)
        nc.vector.reciprocal(inv_sb[:], allred_sb[:])
        nc.vector.tensor_scalar_mul(inv_sb[:], inv_sb[:], 126.0)

        # ---- quantize: transpose [128,128] -> [128,128], uint8 biased ----
        for n in range(nb):
            q_sb = qpool.tile([128, 2, TV], u8, tag="q")
            for gi in range(ngroups):
                t0 = gi * GT
                gidx = n * ngroups + gi
                for half in range(2):
                    ps_t = ps_b.tile([128, 128], bf16, tag="sm")
                    nc.tensor.transpose(
                        ps_t[:],
                        delta_sb[:, gidx, half * 128:(half + 1) * 128],
                        ident_sb[:])
                    # HW's f32->uint8 output conversion rounds to nearest
                    # (CoreSim truncates -- known divergence; HW is truth).
                    for tl in range(GT):
                        nc.vector.tensor_scalar(
                            out=q_sb[:, half, (t0 + tl) * V:(t0 + tl + 1) * V],
                            in0=ps_t[:, 32 * tl:32 * tl + V],
                            scalar1=inv_sb[:], scalar2=128.0,
                            op0=mybir.AluOpType.mult, op1=mybir.AluOpType.add)
            for half in range(2):
                nc.sync.dma_start(out=dq[n, half * 128:(half + 1) * 128, :],
                                  in_=q_sb[:, half, :])

    nc.compile()
    return nc


# ---------------------------------------------------------------------------
# Host-side weight folding
# ---------------------------------------------------------------------------

def fold_weights(inp):
    import ml_dtypes
    bf = ml_dtypes.bfloat16
    s = np.float32(D ** -0.5)
    bn_s = (inp["bn_gamma"] / np.sqrt(inp["bn_var"] + BN_EPS)).astype(np.float32)
    bn_t = (inp["bn_beta"] - inp["bn_mean"] * bn_s).astype(np.float32)
    w = {}
    # [H, C, D]: wphiT[h, c, d] = (W_phi * s)[h*D+d, c]
    w["wphiT"] = np.ascontiguousarray(
        (inp["W_phi"] * s).reshape(H, D, C).transpose(0, 2, 1)).astype(bf)
    w["wpsiT"] = np.ascontiguousarray(
        inp["W_psi"].reshape(H, D, C).transpose(0, 2, 1)).astype(bf)
    w["wgT"] = np.ascontiguousarray(inp["W_g"].T).astype(bf)
    # wdT[h,c,o] = W_d[h,o,c] * bn_s[o]
    w["wdT"] = np.ascontiguousarray(
        (inp["W_d"] * bn_s[None, :, None]).transpose(0, 2, 1)).astype(bf)
    w["bphi"] = np.ascontiguousarray(
        (inp["b_phi"] * s).astype(np.float32).reshape(H, D).T)
    w["bpsi"] = np.ascontiguousarray(
        inp["b_psi"].astype(np.float32).reshape(H, D).T)
    w["bg"] = inp["b_g"].astype(np.float32).reshape(1, O)
    w["bnT"] = (inp["b_d"].sum(axis=0) * bn_s + bn_t).astype(np.float32).reshape(1, O)
    a_base = inp["A_prior"] + np.float32(inp["beta"]) * inp["A_2hop"]  # [H,V,V]
    # abaseT[w, h*V+v] = a_base[h, v, w]
    w["abaseT"] = np.ascontiguousarray(
        a_base.transpose(2, 0, 1).reshape(V, H * V)).astype(np.float32)
    w["lam"] = np.clip(np.float32(inp["lam"]), 0.0, 1.0).reshape(1, 1).astype(np.float32)
    return w


# ---------------------------------------------------------------------------
# SPMD runner (cached jit through bass2jax under axon)
# ---------------------------------------------------------------------------

class SpmdRunner:
    def __init__(self, nc):
        import jax
        import jax.numpy as jnp
        from jax.sharding import Mesh, PartitionSpec as P, NamedSharding
        from jax.experimental.shard_map import shard_map
        import concourse.mybir as mybir
        from concourse import bass2jax

        bass2jax.install_neuronx_cc_hook()
        self.nc = nc
        partition_name = nc.partition_id_tensor.name if nc.partition_id_tensor else None
        in_names, out_names, out_avals = [], [], []
        for alloc in nc.m.functions[0].allocations:
            if not isinstance(alloc, mybir.MemoryLocationSet):
                continue
            name = alloc.memorylocations[0].name
            if alloc.kind == "ExternalInput":
                if name != partition_name:
                    in_names.append(name)
            elif alloc.kind == "ExternalOutput":
                out_names.append(name)
                out_avals.append(jax.core.ShapedArray(
                    tuple(alloc.tensor_shape), mybir.dt.np(alloc.dtype)))
        self.in_names = in_names
        self.out_names = out_names
        self.out_avals = out_avals
        n_params, n_outs = len(in_names), len(out_names)
        bind_in_names = list(in_names) + list(out_names)
        if partition_name is not None:
            bind_in_names.append(partition_name)
        bind_in_names = tuple(bind_in_names)

        def _body(*args):
            operands = list(args)
            if partition_name is not None:
                operands.append(bass2jax.partition_id_tensor())
            outs = bass2jax._bass_exec_p.bind(
                *operands,
                out_avals=tuple(out_avals),
                in_names=bind_in_names,
                out_names=tuple(out_names),
                lowering_input_output_aliases=(),
                sim_require_finite=True,
                sim_require_nnan=True,
                nc=nc,
            )
            return tuple(outs)

        devices = jax.devices()[:NCORES]
        self.mesh = Mesh(np.asarray(devices), ("core",))
        self.sharding = NamedSharding(self.mesh, P("core"))
        in_specs = (P("core"),) * (n_params + n_outs)
        out_specs = (P("core"),) * n_outs
        self.fn = jax.jit(
            shard_map(_body, mesh=self.mesh, in_specs=in_specs,
                      out_specs=out_specs, check_rep=False),
            keep_unused=True,
        )
        # persistent (non-donated) zero-filled output operands, device-resident
        self.zero_bufs = [
            jax.device_put(
                np.zeros((NCORES * a.shape[0], *a.shape[1:]), a.dtype), self.sharding)
            for a in out_avals
        ]
        self._jax = jax

    def __call__(self, global_inputs):
        args = [global_inputs[n] for n in self.in_names]
        outs = self.fn(*args, *self.zero_bufs)
        return dict(zip(self.out_names, outs))


# ---------------------------------------------------------------------------
# Public kernel
# ---------------------------------------------------------------------------

def _kernel_device(inputs):
    import jax
    import ml_dtypes

    if "runner" not in _CACHE:
        _CACHE["runner"] = SpmdRunner(build_nc())
    runner = _CACHE["runner"]

    x = np.asarray(inputs["x"], np.float32)
    xr3 = x.reshape(N, C, T * V)
    x_bf = np.empty((N, C, T * V), ml_dtypes.bfloat16)
    cast_threads = []
    for i in range(4):
        sl = slice(i * (N // 4), (i + 1) * (N // 4))
        th = threading.Thread(
            target=lambda s=sl: x_bf[s].__setitem__(slice(None), xr3[s]))
        th.start()
        cast_threads.append(th)
    for th in cast_threads:
        th.join()
    # kick off the (dominant) x upload before any other host work
    x_dev = jax.device_put(x_bf, runner.sharding)
    w = fold_weights({k: np.asarray(v, np.float32) for k, v in inputs.items()
                      if k != "x"})

    wnames = ("wphiT", "wpsiT", "wgT", "wdT", "bphi", "bpsi", "bg", "bnT",
              "abaseT", "lam")
    cached = _CACHE.get("wdev")
    if cached is None or not all(
            np.array_equal(cached[0][n], w[n]) for n in wnames):
        # stack per-core copies and park them on the devices; weights are
        # tiny but re-uploading ~5MB each call costs ~70ms through the tunnel
        wdev = {}
        for name in wnames:
            arr = w[name]
            stacked = np.broadcast_to(
                arr[None], (NCORES, *arr.shape)).reshape(
                    NCORES * arr.shape[0], *arr.shape[1:])
            wdev[name] = jax.device_put(np.ascontiguousarray(stacked),
                                        runner.sharding)
        _CACHE["wdev"] = (w, wdev)
        cached = _CACHE["wdev"]

    gi = {"x_bf": x_dev}
    gi.update(cached[1])
    outs = runner(gi)
    dq_dev, dmax_dev = outs["dq"], outs["dmax"]

    # overlap D2H of the scale + all dq shards with dequantization
    res = np.empty((N, C, T * V), np.float32)
    shards = sorted(dq_dev.addressable_shards, key=lambda s: s.index[0].start or 0)
    fetched = [None] * NCORES
    dmax_box = [None]

    def fetch_dmax():
        dmax_box[0] = np.asarray(dmax_dev).reshape(NCORES)

    def fetch(i):
        fetched[i] = np.asarray(shards[i].data)

    th_dmax = threading.Thread(target=fetch_dmax)
    th_dmax.start()
    threads = []
    for i in range(NCORES):
        th = threading.Thread(target=fetch, args=(i,))
        th.start()
        threads.append(th)
    th_dmax.join()
    scales = (dmax_box[0] / np.float32(126.0)).astype(np.float32)
    xr = x.reshape(N, C, T * V)
    for i in range(NCORES):
        threads[i].join()
        blk = fetched[i]  # [NB, O, TV] uint8
        s = scales[i]
        sl = slice(i * NB, (i + 1) * NB)
        r = blk.astype(np.float32)
        r -= np.float32(128.0)
        r *= s
        r += xr[sl]
        np.maximum(r, 0.0, out=r)
        res[sl] = r
    return res.reshape(N, C, T, V)


# ---------------------------------------------------------------------------
# Pure-numpy fallback (reference math)
# ---------------------------------------------------------------------------

def _forward_np(x, A_prior, A_2hop, beta, lam, W_phi, b_phi, W_psi, b_psi,
                W_d, b_d, bn_gamma, bn_beta, bn_mean, bn_var, W_g, b_g):
    n, c, t, v = x.shape
    h, d = H, D
    scale = d ** -0.5

    def conv1x1_heads(W, b):
        y = np.einsum('nctv,ec->netv', x, W) + b[None, :, None, None]
        return (y.reshape(n, h, d, t, v).transpose(0, 3, 1, 4, 2)
                 .reshape(n * t, h, v, d))

    phi = conv1x1_heads(W_phi, b_phi)
    psi = conv1x1_heads(W_psi, b_psi)
    logits = np.einsum('bhvd,bhwd->bhvw', phi, psi) * scale
    m = logits.max(axis=-1, keepdims=True)
    e = np.exp(logits - m)
    A_adapt = e / e.sum(axis=-1, keepdims=True)
    lam_c = np.clip(lam, 0.0, 1.0)
    A_final = (A_prior + beta * A_2hop)[None] + lam_c * A_adapt
    feat = x.transpose(0, 2, 3, 1).reshape(n * t, v, c)
    z = np.einsum('bhvw,bwc->bhvc', A_final, feat)
    out = np.einsum('bhvc,hoc->bvo', z, W_d) + b_d.sum(axis=0)
    out = out.reshape(n, t, v, -1).transpose(0, 3, 1, 2)
    inv = 1.0 / np.sqrt(bn_var + BN_EPS)
    out = ((out - bn_mean[None, :, None, None]) * (inv * bn_gamma)[None, :, None, None]
           + bn_beta[None, :, None, None])
    gate = 1.0 / (1.0 + np.exp(-(np.einsum('nctv,oc->notv', x, W_g)
                                 + b_g[None, :, None, None])))
    out = gate * out + x
    return np.maximum(out, 0.0)


def kernel(**inputs) -> np.ndarray:
    try:
        return _kernel_device(inputs)
    except Exception:
        import traceback
        traceback.print_exc()
        args = [np.asarray(inputs[k], np.float32) for k in
                ["x", "A_prior", "A_2hop", "beta", "lam", "W_phi", "b_phi",
                 "W_psi", "b_psi", "W_d", "b_d", "bn_gamma", "bn_beta",
                 "bn_mean", "bn_var", "W_g", "b_g"]]
        return np.asarray(_forward_np(*args), np.float32)
